# revision 15
# baseline (speedup 1.0000x reference)
"""Bahdanau attention Trainium2 kernel (Bass/Tile), 8-core data-parallel.

Problem (per reference):
  B=32, S=2048, ENC=DEC=1024, ATTN=512, fp32 inputs.
  proj_h = einsum('bse,ae->bsa', enc, W_h)
  proj_s = einsum('bd,ad->ba', dec, W_s)
  e      = tanh(proj_h + proj_s[:, None, :])
  score  = einsum('bsa,oa->bs', e, v);  score = where(mask, score, -inf)
  w      = softmax(score, axis=1)
  ctx    = einsum('bs,bse->be', w, enc)
  returns (ctx, w)

Sharding: batch over 8 cores (4 per core), weights replicated.

Per-core design:
  - enc loaded HBM->SBUF with fp32->bf16 cast in the DMA (SWDGE), natural
    layout [s%128, (s_tile, e)].
  - enc^T built with the xbar DMA transpose (bf16): [e%128, (e_chunk, s)].
  - proj_h^T[a, s] accumulated on PE over 8 e-chunks (bf16, fp32 PSUM);
    tanh+bias (proj_s^T per-partition) fused in one ScalarE activation,
    output bf16.
  - score^T via PE: lhsT=v chunk [a,1], rhs=e[a, s-slice]; psum [1, 512].
  - softmax without max-subtraction (|score| <= sum|v| ~ 20, exp is safe in
    fp32); scores bounced through DRAM to a [128, 16] layout (s = j*128+p);
    Z reduced on-partition then across partitions with a ones-matmul.
  - context = sum_s exp_s * enc[s, :] on PE using unnormalized bf16 exp as
    lhsT [s-chunk, 1] over natural-layout enc; scaled by 1/Z on partition 0.
  - attn weights normalized with 1/Z broadcast via a DRAM bounce and written
    back through a DRAM reorder.
  - Phases are software-pipelined: batch b's softmax tail + context are
    emitted after batch b+1's main matmuls so the PE never waits on the
    softmax critical path.
"""

import os
import sys
from contextlib import ExitStack

for _p in ("/opt/trn_rl_repo", "/root/.axon_site/_ro/trn_rl_repo"):
    if os.path.isdir(_p) and _p not in sys.path:
        sys.path.append(_p)

import numpy as np

import concourse.bacc as bacc
import concourse.tile as tile
from concourse import mybir
from concourse.bass_utils import run_bass_kernel_spmd, axon_active

N_CORES = 8
B, S, ENC, DEC, ATTN = 32, 2048, 1024, 1024, 512
BL = B // N_CORES          # batches per core = 4
ST = S // 128              # s tiles = 16
EC = ENC // 128            # e chunks = 8
AT = ATTN // 128           # a tiles = 4
SL = S // 512              # 512-wide s slices = 4

F32 = mybir.dt.float32
BF16 = mybir.dt.bfloat16
U8 = mybir.dt.uint8
TANH = mybir.ActivationFunctionType.Tanh
EXP = mybir.ActivationFunctionType.Exp
AXX = mybir.AxisListType.X

_CACHE = {}


def _build():
    nc = bacc.Bacc("TRN2", target_bir_lowering=False, debug=not axon_active(),
                   num_devices=N_CORES)

    enc_t = nc.dram_tensor("enc", [BL, S, ENC], F32, kind="ExternalInput").ap()
    dec_t = nc.dram_tensor("dec", [BL, DEC], F32, kind="ExternalInput").ap()
    mask_t = nc.dram_tensor("mask", [BL, S], U8, kind="ExternalInput").ap()
    wh_t = nc.dram_tensor("W_h", [ATTN, ENC], F32, kind="ExternalInput").ap()
    ws_t = nc.dram_tensor("W_s", [ATTN, DEC], F32, kind="ExternalInput").ap()
    v_t = nc.dram_tensor("v", [1, ATTN], F32, kind="ExternalInput").ap()

    ctx_t = nc.dram_tensor("ctx", [BL, ENC], F32, kind="ExternalOutput").ap()
    attn_t = nc.dram_tensor("attn", [BL, S], F32, kind="ExternalOutput").ap()

    score_scr = nc.dram_tensor("score_scr", [BL, S], F32).ap()
    z_scr = nc.dram_tensor("z_scr", [BL, 1], F32).ap()

    with tile.TileContext(nc) as tc, ExitStack() as es:
        _emit(es, tc, enc_t, dec_t, mask_t, wh_t, ws_t, v_t,
              ctx_t, attn_t, score_scr, z_scr)

    nc.compile()
    return nc


def _emit(es, tc, enc_t, dec_t, mask_t, wh_t, ws_t, v_t,
          ctx_t, attn_t, score_scr, z_scr):
    nc = tc.nc

    const = es.enter_context(tc.tile_pool(name="const", bufs=1))
    enc_pool = es.enter_context(tc.tile_pool(name="encp", bufs=2))
    encT_pool = es.enter_context(tc.tile_pool(name="encTp", bufs=2))
    ebf_pool = es.enter_context(tc.tile_pool(name="ebf", bufs=8))
    sm_pool = es.enter_context(tc.tile_pool(name="sm", bufs=2))
    big1_pool = es.enter_context(tc.tile_pool(name="big1", bufs=1))
    ph_psum = es.enter_context(tc.tile_pool(name="php", bufs=2, space="PSUM"))
    sc_psum = es.enter_context(tc.tile_pool(name="scp", bufs=2, space="PSUM"))
    ctx_psum = es.enter_context(tc.tile_pool(name="ctxp", bufs=2, space="PSUM"))
    z_psum = es.enter_context(tc.tile_pool(name="zp", bufs=2, space="PSUM"))

    # ---------------- setup: weights / decoder / v ----------------
    ones_f32 = const.tile([128, 1], F32)
    nc.vector.memset(ones_f32, 1.0)

    whT = const.tile([128, EC, ATTN], BF16, tag="whT")
    wsT = const.tile([128, EC, ATTN], BF16, tag="wsT")
    with tc.tile_pool(name="stage", bufs=1) as stage:
        for w_dram, wT in ((wh_t, whT), (ws_t, wsT)):
            w_nat = stage.tile([128, AT, ENC], F32, tag="wnat")
            nc.sync.dma_start(out=w_nat, in_=w_dram.rearrange(
                "(t p) e -> p t e", p=128))
            w_nat_bf = stage.tile([128, AT, ENC], BF16, tag="wnatbf")
            nc.vector.tensor_copy(out=w_nat_bf, in_=w_nat)
            for t in range(AT):
                # in [a=128, e=1024] -> out[p_e, c, l_a] = w[a=t*128+l, e=c*128+p]
                nc.sync.dma_start(out=wT[:, :, t * 128:(t + 1) * 128],
                                  in_=w_nat_bf[:, t, :], transpose=True)

        decT_f32 = stage.tile([128, EC, BL], F32, tag="decT32")
        for bb in range(BL):
            nc.gpsimd.dma_start(out=decT_f32[:, :, bb],
                                in_=dec_t[bb].rearrange("(c p) -> p c", p=128))
        decT = const.tile([128, EC, BL], BF16, tag="decT")
        nc.vector.tensor_copy(out=decT, in_=decT_f32)

        v_f32 = stage.tile([128, AT], F32, tag="v32")
        nc.gpsimd.dma_start(out=v_f32,
                            in_=v_t[0].rearrange("(c p) -> p c", p=128))
        vT = const.tile([128, AT], BF16, tag="vT")
        nc.vector.tensor_copy(out=vT, in_=v_f32)

    # proj_s^T[a, b] for all local batches: accumulate over e chunks
    proj_s = const.tile([128, AT, BL], F32, tag="projs")
    for t in range(AT):
        ps = ph_psum.tile([128, BL], F32, tag="ph")
        for c in range(EC):
            nc.tensor.matmul(ps, lhsT=wsT[:, c, t * 128:(t + 1) * 128],
                             rhs=decT[:, c, :], start=(c == 0), stop=(c == EC - 1))
        nc.scalar.copy(out=proj_s[:, t, :], in_=ps)

    # ---------------- pipelined per-batch phases ----------------
    # stash[b] holds cross-phase tiles for batch b
    stash = {}

    for it in range(BL + 1):
        if it < BL:
            b = it
            st = {}
            stash[b] = st

            # --- load enc (cast fp32->bf16 in DMA), 4 t-groups of 4 tiles.
            # Partition assignment: s = p*16 + t, so each partition reads one
            # contiguous 16-row block of enc -> perfect DMA descriptors, and
            # enc_nat[:, t, :] pairs with weight column t in the context MM.
            enc_nat = enc_pool.tile([128, ST, ENC], BF16, tag="encnat")
            enc_src = enc_t[b].rearrange("(p t) e -> p t e", t=ST)
            for g in range(4):
                nc.gpsimd.dma_start(out=enc_nat[:, g * 4:(g + 1) * 4, :],
                                    in_=enc_src[:, g * 4:(g + 1) * 4, :])
            st["enc_nat"] = enc_nat

            # --- xbar transpose to [e%128, (c, q)] where the free position
            # q = t*128 + l holds the row with s = l*16 + t (a fixed
            # permutation of s; everything through the score stays in
            # q-order and is elementwise in s, so only the score vector
            # needs re-permuting afterwards).
            encT = encT_pool.tile([128, EC, S], BF16, tag="encT")
            for t in range(ST):
                nc.sync.dma_start(out=encT[:, :, t * 128:(t + 1) * 128],
                                  in_=enc_nat[:, t, :], transpose=True)

            # --- main matmul + tanh + score, one 512-wide s-slice at a time.
            # The score PSUM slice is in q-order (q = t*128 + l, s = l*16+t);
            # the PSUM->SBUF copy un-permutes it straight into s-order.
            score_sord = big1_pool.tile([1, S], F32, tag="scoresord")
            for sl in range(SL):
                ssl = slice(sl * 512, (sl + 1) * 512)
                ebfs = []
                for t in range(AT):
                    ph = ph_psum.tile([128, 512], F32, tag="ph")
                    for c in range(EC):
                        nc.tensor.matmul(ph, lhsT=whT[:, c, t * 128:(t + 1) * 128],
                                         rhs=encT[:, c, ssl],
                                         start=(c == 0), stop=(c == EC - 1))
                    ebf = ebf_pool.tile([128, 512], BF16, tag="ebf")
                    nc.scalar.activation(out=ebf, in_=ph, func=TANH,
                                         bias=proj_s[:, t, b:b + 1], scale=1.0)
                    ebfs.append(ebf)
                sc = sc_psum.tile([1, 512], F32, tag="sc")
                for t in range(AT):
                    nc.tensor.matmul(sc, lhsT=vT[:, t:t + 1], rhs=ebfs[t],
                                     start=(t == 0), stop=(t == AT - 1))
                sord_view = score_sord.rearrange("o (l t) -> o l t", t=ST)
                nc.vector.tensor_copy(
                    out=sord_view[:, :, sl * 4:(sl + 1) * 4],
                    in_=sc.rearrange("o (t l) -> o l t", t=4))

            # --- softmax part 1 (no PE): bounce through DRAM into [128, 16]
            #     (s = p*16 + j, contiguous per partition), mask, exp, sums
            nc.gpsimd.dma_start(out=score_scr[b:b + 1, :], in_=score_sord)
            score_T = sm_pool.tile([128, ST], F32, tag="scoreT")
            nc.gpsimd.dma_start(out=score_T,
                                in_=score_scr[b].rearrange("(p j) -> p j", j=ST))
            mask_T = sm_pool.tile([128, ST], U8, tag="maskT")
            nc.gpsimd.dma_start(out=mask_T,
                                in_=mask_t[b].rearrange("(p j) -> p j", j=ST))
            m_f32 = sm_pool.tile([128, ST], F32, tag="mf32")
            nc.vector.tensor_copy(out=m_f32, in_=mask_T)
            # score_m = score*m + (m-1)*1e30  (all-ones mask -> identity)
            mneg = sm_pool.tile([128, ST], F32, tag="mneg")
            nc.vector.tensor_scalar(out=mneg, in0=m_f32, scalar1=1.0,
                                    scalar2=1e30, op0=mybir.AluOpType.subtract,
                                    op1=mybir.AluOpType.mult)
            score_m = sm_pool.tile([128, ST], F32, tag="scorem")
            nc.vector.tensor_mul(out=score_m, in0=score_T, in1=m_f32)
            nc.vector.tensor_add(out=score_m, in0=score_m, in1=mneg)

            w_exp = sm_pool.tile([128, ST], F32, tag="wexp")
            nc.scalar.activation(out=w_exp, in_=score_m, func=EXP)
            w_exp_bf = sm_pool.tile([128, ST], BF16, tag="wexpbf")
            nc.vector.tensor_copy(out=w_exp_bf, in_=w_exp)
            rowsum = sm_pool.tile([128, 1], F32, tag="rowsum")
            nc.vector.reduce_sum(out=rowsum, in_=w_exp, axis=AXX)
            st["w_exp"] = w_exp
            st["w_exp_bf"] = w_exp_bf
            st["rowsum"] = rowsum

        if it > 0:
            b = it - 1
            st = stash[b]

            # --- softmax part 2: Z across partitions, 1/Z, outputs
            zp = z_psum.tile([1, 1], F32, tag="z")
            nc.tensor.matmul(zp, lhsT=ones_f32, rhs=st["rowsum"],
                             start=True, stop=True)
            rz = sm_pool.tile([1, 1], F32, tag="rz")
            z_sb = sm_pool.tile([1, 1], F32, tag="zsb")
            nc.vector.tensor_copy(out=z_sb, in_=zp)
            nc.vector.reciprocal(out=rz, in_=z_sb)

            # --- context: unnormalized sum on PE, then scale by 1/Z
            cps = [ctx_psum.tile([1, 512], F32, tag="ctx", name=f"ctx_{b}_{h}")
                   for h in range(2)]
            enc_nat = st["enc_nat"]
            for j in range(ST):
                for h in range(2):
                    nc.tensor.matmul(cps[h], lhsT=st["w_exp_bf"][:, j:j + 1],
                                     rhs=enc_nat[:, j, h * 512:(h + 1) * 512],
                                     start=(j == 0), stop=(j == ST - 1))
            ctx_sb = big1_pool.tile([1, ENC], F32, tag="ctxsb")
            for h in range(2):
                nc.vector.tensor_scalar_mul(out=ctx_sb[:, h * 512:(h + 1) * 512],
                                            in0=cps[h], scalar1=rz)
            nc.sync.dma_start(out=ctx_t[b:b + 1, :], in_=ctx_sb)

            # --- attn weights: 1/Z broadcast via DRAM bounce, normalize,
            #     direct store (s = p*16 + j is contiguous per partition)
            nc.gpsimd.dma_start(out=z_scr[b:b + 1, :], in_=rz)
            rz_bc = sm_pool.tile([128, 1], F32, tag="rzbc")
            nc.gpsimd.dma_start(out=rz_bc, in_=z_scr[b:b + 1, :].to_broadcast((128, 1)))
            attn_f = sm_pool.tile([128, ST], F32, tag="attnf")
            nc.vector.tensor_scalar_mul(out=attn_f, in0=st["w_exp"], scalar1=rz_bc)
            nc.gpsimd.dma_start(out=attn_t[b].rearrange("(p j) -> p j", j=ST),
                                in_=attn_f)
            del stash[b]


def _get_nc():
    if "nc" not in _CACHE:
        _CACHE["nc"] = _build()
    return _CACHE["nc"]


def kernel(encoder_outputs, decoder_hidden, mask, W_h, W_s, v):
    enc = np.ascontiguousarray(np.asarray(encoder_outputs, dtype=np.float32))
    dec = np.ascontiguousarray(np.asarray(decoder_hidden, dtype=np.float32))
    msk = np.ascontiguousarray(np.asarray(mask)).astype(np.uint8)
    wh = np.ascontiguousarray(np.asarray(W_h, dtype=np.float32))
    ws = np.ascontiguousarray(np.asarray(W_s, dtype=np.float32))
    vv = np.ascontiguousarray(np.asarray(v, dtype=np.float32))

    nc = _get_nc()
    in_maps = []
    for i in range(N_CORES):
        sl = slice(i * BL, (i + 1) * BL)
        in_maps.append({"enc": enc[sl], "dec": dec[sl], "mask": msk[sl],
                        "W_h": wh, "W_s": ws, "v": vv})

    trace = bool(int(os.environ.get("KERNEL_PROFILE", "0")))
    res = run_bass_kernel_spmd(nc, in_maps, list(range(N_CORES)), trace=trace)
    _CACHE["last_exec_ns"] = res.exec_time_ns
    _CACHE["last_profile"] = res.profile_json

    ctx = np.concatenate([res.results[i]["ctx"] for i in range(N_CORES)], axis=0)
    attn = np.concatenate([res.results[i]["attn"] for i in range(N_CORES)], axis=0)
    return ctx.astype(np.float32), attn.astype(np.float32)
